# revision 14
# baseline (speedup 1.0000x reference)
"""BasicGCN (4x GCNConv+BN+ReLU, mean/max/sum pool, MLP) on 8 TRN2 NeuronCores.

Strategy:
  - Graphs are assigned to cores (32 graphs/core), each graph gets a fixed
    512-column slot; nodes live in their graph's slot => pooling is uniform
    free-axis reduces, no cross-core graphs.
  - Per layer: z = h @ W on PE (feature-major), y = dinv*z, PE-transpose to
    node-major fp16, DMA out, AllGather in 8 stripe pieces (16384-row blocks,
    inside int16 gather reach).
  - Message aggregation: per (256-col call-group cg, src-block b) dma_gather
    of the edge sources (dst-sorted slot list, cross-core-maxed cell sizes,
    16-aligned only => ~1.14x slot inflation), spread round-robin over the
    4 SWDGE queues so descriptor rings drain concurrently (~4x gather rate);
    256-wide one-hot selector columns built on DVE via is_equal(iota, dcol)
    in fp16, PE matmuls (stationary = gathered fp16 chunk, moving = selector
    [k,256]) accumulate scatter-add results in a fp32 [128,256] PSUM region
    per cg.
  - Block order rotated by cg so early call-groups don't head-of-line block
    on late AllGather pieces.
  - Next layer's dense phase (z/transpose/AG stripe k) is emitted right after
    the evictions that produce its hT columns, hiding layer boundaries.
  - Eviction fuses +self_loop, *dinv, BN affine (A,B folded) and ReLU.
  - Pool: 512-col reduces (sum/max) + one merged AllGather + pad correction
    + MLP on PE.
"""
import math
import numpy as np
import ml_dtypes

from concourse import bass, mybir, bacc, tile
from concourse import library_config

F32 = np.float32
FP16 = np.float16
H = 128          # feature dim == partitions
EPS = 1e-5


# ============================== host planning ==============================

class Plan:
    pass


def build_plan(edge_index, batch, N, B, n_cores=8, slot=512, L=4):
    p = Plan()
    assert B % n_cores == 0
    gpc = B // n_cores                 # graphs per core
    P_own = slot * gpc                 # columns per core
    assert P_own % 1024 == 0
    STRIPE = P_own // 8                # rows per AG piece
    assert STRIPE % 128 == 0
    BLOCK = STRIPE * n_cores           # rows per gather block
    assert BLOCK <= 32767
    NCG = P_own // 256                 # call groups (256 cols each)

    src = np.asarray(edge_index[0]).astype(np.int64)
    dst = np.asarray(edge_index[1]).astype(np.int64)
    batch = np.asarray(batch)

    counts = np.bincount(batch, minlength=B).astype(np.int64)
    assert counts.max() <= slot, (counts.max(), slot)
    starts = np.concatenate([[0], np.cumsum(counts)[:-1]])

    # node -> (core, col)
    gr_of = batch.astype(np.int64)
    core_of = gr_of // gpc
    col_of = (gr_of % gpc) * slot + (np.arange(N) - starts[gr_of])
    assert (col_of < P_own).all()

    # gid: global padded row id (stripe-major)
    stripe_of = col_of // STRIPE
    gid = BLOCK * stripe_of + STRIPE * core_of + (col_of % STRIPE)

    deg = np.bincount(dst, minlength=N).astype(np.float64) + 1.0
    dinv = (1.0 / np.sqrt(deg)).astype(F32)

    # per-edge attributes (dst side)
    e_core = core_of[dst]
    e_col = col_of[dst]
    e_cg = e_col // 256
    e_blk = gid[src] // BLOCK
    e_idx = gid[src] % BLOCK

    # cell counts maxed over cores (SPMD: one program, shared static sizes)
    NBLK = 8
    cell = np.zeros((n_cores, NCG, NBLK), np.int64)
    np.add.at(cell, (e_core, e_cg, e_blk), 1)
    ncell = cell.max(axis=0)            # [NCG, NBLK]

    # paired gather calls: call (j, b) spans cgs (2j, 2j+1); the second
    # region starts 32-aligned so matmul pieces stay on legal base partitions
    NPAIR = NCG // 2
    lenA = ncell[0::2, :]                     # [NPAIR, NBLK]
    lenB = ncell[1::2, :]
    obnd = ((lenA + 31) // 32) * 32           # region-B start within call
    call_len = ((obnd + lenB + 15) // 16) * 16
    call_off = np.zeros((NPAIR, NBLK), np.int64)
    call_ccol = np.zeros((NPAIR, NBLK), np.int64)
    off = 0
    ccol = 0
    for j in range(NPAIR):
        for b in range(NBLK):
            call_off[j, b] = off
            call_ccol[j, b] = ccol
            off += int(call_len[j, b])
            ccol += (int(call_len[j, b]) + 127) // 128
    TOT = off
    TOTCOL = ccol
    CCMAX = int(max((int(call_len[j, b]) + 127) // 128
                    for j in range(NPAIR) for b in range(NBLK)))

    # per-core slot data: slots per (cg, b) sorted by dst col
    idx_all = np.zeros((n_cores, TOT), np.int16)
    dcol_all = np.full((n_cores, TOT), -1.0, F32)
    order = np.lexsort((e_col, e_blk, e_cg, e_core))
    eo_core, eo_cg, eo_blk = e_core[order], e_cg[order], e_blk[order]
    eo_idx, eo_col = e_idx[order], e_col[order]
    key = (eo_core * NCG + eo_cg) * NBLK + eo_blk
    uq, st = np.unique(key, return_index=True)
    st = list(st) + [len(key)]
    for u, s0, s1 in zip(uq, st[:-1], st[1:]):
        b = int(u) % NBLK
        cg = (int(u) // NBLK) % NCG
        c = int(u) // (NBLK * NCG)
        n = s1 - s0
        j, r = cg // 2, cg % 2
        pos = int(call_off[j, b]) + (int(obnd[j, b]) if r else 0)
        idx_all[c, pos:pos + n] = eo_idx[s0:s1].astype(np.int16)
        dcol_all[c, pos:pos + n] = (eo_col[s0:s1] % 256).astype(F32)

    # pieces per call (j, b): (ccol, base_part, k, region); bases limited to
    # {0,32,64,96} with per-base K caps (PE sub-quadrant addressing)
    CAP = {0: 128, 32: 32, 64: 64, 96: 32}
    pieces = {}
    for j in range(NPAIR):
        for b in range(NBLK):
            pl = []
            for r, (s0, n) in enumerate([(0, int(lenA[j, b])),
                                         (int(obnd[j, b]), int(lenB[j, b]))]):
                o = int(s0)
                while n > 0:
                    pp = o % 128
                    k = min(CAP[pp], n)
                    pl.append((o // 128, pp, k, r))
                    o += k
                    n -= k
            pieces[(j, b)] = pl

    # block visit order per pair (rotated so pair j starts on AG piece j%8)
    border = {j: [(j + k) % NBLK for k in range(NBLK)] for j in range(NPAIR)}
    # first/last (b, piece i) per cg (psum region) in rotated issue order
    flags = {}
    for cg in range(NCG):
        j, r = cg // 2, cg % 2
        seq = [(b, i) for b in border[j]
               for i, pc in enumerate(pieces[(j, b)]) if pc[3] == r]
        flags[cg] = (seq[0], seq[-1]) if seq else (None, None)

    p.__dict__.update(locals())
    return p


def prepare_inputs(p, x, conv_ws, conv_bs, bn_gamma, bn_beta, bn_mean, bn_var,
                   fc1_w, fc1_b, fc2_w, fc2_b, fc3_w, fc3_b):
    """Build per-core in_maps (list of dicts of np arrays)."""
    n_cores, P_own, N, B, L = p.n_cores, p.P_own, p.N, p.B, p.L
    A = (bn_gamma / np.sqrt(bn_var + EPS)).astype(F32)      # [L,H]
    Bv = ((conv_bs - bn_mean) * A + bn_beta).astype(F32)    # [L,H]
    AB = np.zeros((H, 2 * L), F32)
    for l in range(L):
        AB[:, 2 * l] = A[l]
        AB[:, 2 * l + 1] = Bv[l]
    reluB_last = np.maximum(Bv[L - 1], 0.0)                 # [H]
    padn = (p.slot - p.counts).astype(F32)                  # [B]
    padcorr = np.outer(reluB_last, padn).astype(F32)        # [H, B]
    cntinv = np.tile((1.0 / np.maximum(p.counts, 1.0)).astype(F32), (H, 1))

    Wl = np.zeros((H, L, H), F32)
    for l in range(L):
        Wl[:, l, :] = conv_ws[l]
    fc1 = np.zeros((H, 3, H), F32)
    for k in range(3):
        fc1[:, k, :] = fc1_w[k * H:(k + 1) * H, :]
    iota = np.tile(np.arange(256, dtype=F32)[None, None, :],
                   (128, p.CCMAX, 1)).astype(FP16)
    ident = np.eye(128, dtype=F32)

    in_maps = []
    for c in range(n_cores):
        m = p.core_of == c
        hT0 = np.zeros((H, P_own), F32)
        hT0[:, p.col_of[m]] = np.asarray(x)[m].T
        dr = np.zeros((H, P_own), F32)
        dr[0, p.col_of[m]] = p.dinv[m]
        dr[:, :] = dr[0][None, :]
        # idx wrapped: slot i -> [i%16, i//16], replicated to 128 partitions
        iw = p.idx_all[c].reshape(-1, 16).T            # [16, TOT/16]
        iw = np.tile(iw, (8, 1)).astype(np.int16)      # [128, TOT/16]
        # dcol: slot (128*ccol + pp) -> [pp, ccol]; calls packed by ccol
        dc = np.full((128, p.TOTCOL, 1), -1.0, F32)
        for jj in range(p.NPAIR):
            for b in range(p.NBLK):
                o = int(p.call_off[jj, b])
                ln = int(p.call_len[jj, b])
                cc0 = int(p.call_ccol[jj, b])
                seg = np.full(((ln + 127) // 128) * 128, -1.0, F32)
                seg[:ln] = p.dcol_all[c, o:o + ln]
                dc[:, cc0:cc0 + len(seg) // 128, 0] = seg.reshape(-1, 128).T
        in_maps.append(dict(
            hT0=hT0, dinv_rep=dr, idx=iw, dcol=dc.astype(FP16),
            iota=iota, ident=ident, Wl=Wl, AB=AB,
            padcorr=padcorr, cntinv=cntinv, fc1w=fc1,
            fc1b=np.asarray(fc1_b, F32).reshape(H, 1),
            fc2w=np.asarray(fc2_w, F32),
            fc2b=np.asarray(fc2_b, F32).reshape(H // 2, 1),
            fc3w=np.asarray(fc3_w, F32).reshape(H // 2, 1),
        ))
    return in_maps, float(np.asarray(fc3_b).reshape(-1)[0])


# ============================== device program =============================

def build_nc(p, fc3b_val, debug=False, linearize=False):
    n_cores, P_own, L = p.n_cores, p.P_own, p.L
    STRIPE, BLOCK, NCG, NBLK = p.STRIPE, p.BLOCK, p.NCG, p.NBLK
    B = p.B
    gpc = p.gpc
    CCMAX = p.CCMAX
    f32, fp16, i16 = mybir.dt.float32, mybir.dt.float16, mybir.dt.int16

    nc = bacc.Bacc("TRN2", target_bir_lowering=False, debug=debug,
                   num_devices=n_cores, num_swdge_queues=4)
    groups = [list(range(n_cores))]

    # dram parameters
    P = {}
    P["hT0"] = nc.dram_tensor("hT0", [H, P_own], f32, kind="ExternalInput")
    P["dinv_rep"] = nc.dram_tensor("dinv_rep", [H, P_own], f32,
                                   kind="ExternalInput")
    P["idx"] = nc.dram_tensor("idx", [128, p.TOT // 16], i16,
                              kind="ExternalInput")
    P["dcol"] = nc.dram_tensor("dcol", [128, p.TOTCOL, 1], fp16,
                               kind="ExternalInput")
    P["iota"] = nc.dram_tensor("iota", [128, CCMAX, 256], fp16,
                               kind="ExternalInput")
    P["ident"] = nc.dram_tensor("ident", [128, 128], f32,
                                kind="ExternalInput")
    P["Wl"] = nc.dram_tensor("Wl", [H, L, H], f32, kind="ExternalInput")
    P["AB"] = nc.dram_tensor("AB", [H, 2 * L], f32, kind="ExternalInput")
    P["padcorr"] = nc.dram_tensor("padcorr", [H, B], f32,
                                  kind="ExternalInput")
    P["cntinv"] = nc.dram_tensor("cntinv", [H, B], f32, kind="ExternalInput")
    P["fc1w"] = nc.dram_tensor("fc1w", [H, 3, H], f32, kind="ExternalInput")
    P["fc1b"] = nc.dram_tensor("fc1b", [H, 1], f32, kind="ExternalInput")
    P["fc2w"] = nc.dram_tensor("fc2w", [H, H // 2], f32, kind="ExternalInput")
    P["fc2b"] = nc.dram_tensor("fc2b", [H // 2, 1], f32, kind="ExternalInput")
    P["fc3w"] = nc.dram_tensor("fc3w", [H // 2, 1], f32, kind="ExternalInput")
    out_t = nc.dram_tensor("out", [1, B], f32, kind="ExternalOutput")

    # dram internals
    ynode_d = nc.dram_tensor("ynode_d", [P_own, H], fp16)
    yfull = [nc.dram_tensor(f"yfull{v}", [BLOCK * 8, H], fp16,
                            addr_space="Shared") for v in range(2)]
    gpool_in = nc.dram_tensor("gpool_in", [H, 2 * gpc], f32)
    gpool_out = nc.dram_tensor("gpool_out", [H * n_cores, 2 * gpc], f32,
                               addr_space="Shared")

    L16MAX = int(p.call_len.max() // 16)
    NT = STRIPE // 128           # transposes per stripe

    import contextlib
    with tile.TileContext(nc, linearize=linearize) as tc, \
            contextlib.ExitStack() as octx:
        nc.gpsimd.load_library(library_config.mlp)
        cpool = octx.enter_context(tc.tile_pool(name="consts", bufs=1))
        with contextlib.ExitStack() as ctx:
            hT = cpool.tile([H, P_own], f32)
            dinv = cpool.tile([H, P_own], f32)
            dcol_sb = cpool.tile([128, p.TOTCOL, 1], fp16)
            iota = cpool.tile([128, CCMAX, 256], fp16)
            ident = cpool.tile([128, 128], f32)
            W_sb = cpool.tile([H, L, H], f32)
            AB_sb = cpool.tile([H, 2 * L], f32)
            for name, t in [("hT0", hT), ("dinv_rep", dinv),
                            ("dcol", dcol_sb), ("iota", iota),
                            ("ident", ident), ("Wl", W_sb), ("AB", AB_sb)]:
                nc.sync.dma_start(t[:], P[name][:])

            ynpool = ctx.enter_context(tc.tile_pool(name="ynp", bufs=2))
            stpool = ctx.enter_context(tc.tile_pool(name="stp", bufs=6))
            sgpool = ctx.enter_context(tc.tile_pool(name="sgp", bufs=4))
            ixpool = ctx.enter_context(tc.tile_pool(name="ixp", bufs=6))
            evpool = ctx.enter_context(tc.tile_pool(name="evp", bufs=4))
            zpool = ctx.enter_context(
                tc.tile_pool(name="zp", bufs=2, space="PSUM"))
            tpool = ctx.enter_context(
                tc.tile_pool(name="tp", bufs=2, space="PSUM"))
            apool = ctx.enter_context(
                tc.tile_pool(name="ap", bufs=4, space="PSUM"))

            def emit_z_stripe(l, k):
                """z = W^T h, y = dinv*z (in place), transpose, DMA, AG."""
                yf = yfull[l % 2]
                base = STRIPE * k
                o = 0
                while o < STRIPE:
                    w = min(512, STRIPE - o)
                    zp = zpool.tile([128, 512], f32, space="PSUM",
                                    tag="zp", name="zpt")
                    cols = slice(base + o, base + o + w)
                    nc.tensor.matmul(zp[:, :w], lhsT=W_sb[:, l, :],
                                     rhs=hT[:, cols], start=True, stop=True)
                    nc.vector.tensor_tensor(
                        out=hT[:, cols], in0=zp[:, :w],
                        in1=dinv[:, cols], op=mybir.AluOpType.mult)
                    o += w
                yn_sb = ynpool.tile([128, NT, 128], fp16, tag="yn",
                                    name="ynt")
                for t in range(NT):
                    tp = tpool.tile([128, 128], f32, space="PSUM",
                                    tag="tp", name="tpt")
                    cols = slice(base + 128 * t, base + 128 * (t + 1))
                    nc.tensor.transpose(tp[:], hT[:, cols], ident[:])
                    nc.scalar.activation(
                        out=yn_sb[:, t, :], in_=tp[:],
                        func=mybir.ActivationFunctionType.Copy)
                dview = ynode_d[base:base + STRIPE, :].rearrange(
                    "(t q) f -> q t f", q=128)
                nc.sync.dma_start(out=dview, in_=yn_sb[:])
                nc.gpsimd.collective_compute(
                    "AllGather", mybir.AluOpType.bypass,
                    replica_groups=groups,
                    ins=[ynode_d[base:base + STRIPE, :]],
                    outs=[yf[BLOCK * k:BLOCK * (k + 1), :]])

            qctr = [0]

            def emit_agg_pair(l, j):
                """Gather+scatter-add 512 dst cols (cgs 2j,2j+1), evict."""
                yf = yfull[l % 2]
                ps = [apool.tile([128, 256], f32, space="PSUM",
                                 tag="agg", name="aggt") for _ in range(2)]
                fl = [p.flags[2 * j], p.flags[2 * j + 1]]
                for b in p.border[j]:
                    ln = int(p.call_len[j, b])
                    if ln == 0:
                        continue
                    cc0 = int(p.call_ccol[j, b])
                    ncc = (ln + 127) // 128
                    o16 = int(p.call_off[j, b]) // 16
                    ixt = ixpool.tile([128, L16MAX], i16, tag="ix",
                                      name="ixt")
                    nc.sync.dma_start(ixt[:, :ln // 16],
                                      P["idx"][:, o16:o16 + ln // 16])
                    stag = stpool.tile([128, CCMAX, 128], fp16, tag="st",
                                       name="stagt")
                    # HW limit: <=1024 idxs (64 descs/engine) per call
                    for o in range(0, ln, 1024):
                        sub = min(1024, ln - o)
                        nc.gpsimd.dma_gather(
                            stag[:, o // 128:(o + sub + 127) // 128, :],
                            yf[BLOCK * b:BLOCK * (b + 1), :],
                            ixt[:, o // 16:(o + sub) // 16],
                            sub, sub, 128, single_packet=True,
                            queue_num=qctr[0] % 4)
                        qctr[0] += 1
                    st = sgpool.tile([128, CCMAX, 256], fp16, tag="sg",
                                     name="sgt")
                    nc.vector.tensor_tensor(
                        out=st[:, :ncc, :], in0=iota[:, :ncc, :],
                        in1=dcol_sb[:, cc0:cc0 + ncc, :]
                            .to_broadcast([128, ncc, 256]),
                        op=mybir.AluOpType.is_equal)
                    for i, (cc, pp, kk, r) in enumerate(p.pieces[(j, b)]):
                        first, last = fl[r]
                        nc.tensor.matmul(
                            ps[r][:], lhsT=stag[pp:pp + kk, cc, :],
                            rhs=st[pp:pp + kk, cc, :],
                            start=(first == (b, i)), stop=(last == (b, i)),
                            tile_position=((96, 0) if pp == 96 else None))
                for r in range(2):
                    cg = 2 * j + r
                    first = fl[r][0]
                    cols = slice(256 * cg, 256 * (cg + 1))
                    t2 = evpool.tile([128, 256], f32, tag="ev2", name="ev2t")
                    if first is not None:
                        t1 = evpool.tile([128, 256], f32, tag="ev1",
                                         name="ev1t")
                        nc.vector.tensor_tensor(
                            out=t1[:], in0=ps[r][:], in1=hT[:, cols],
                            op=mybir.AluOpType.add)
                        nc.vector.tensor_tensor(
                            out=t2[:], in0=t1[:], in1=dinv[:, cols],
                            op=mybir.AluOpType.mult)
                    else:
                        nc.vector.tensor_tensor(
                            out=t2[:], in0=hT[:, cols], in1=dinv[:, cols],
                            op=mybir.AluOpType.mult)
                    nc.scalar.activation(
                        out=hT[:, cols], in_=t2[:],
                        func=mybir.ActivationFunctionType.Relu,
                        scale=AB_sb[:, 2 * l:2 * l + 1],
                        bias=AB_sb[:, 2 * l + 1:2 * l + 2])

            # layer 0 dense phase up front; later layers interleave with the
            # previous layer's aggregation (stripe k ready after cg 4k+3)
            for k in range(8):
                emit_z_stripe(0, k)
            for l in range(L):
                knext = 0            # next stripe of layer l+1 to emit
                for j in range(p.NPAIR):
                    emit_agg_pair(l, j)
                    while (l < L - 1 and knext < 8
                           and (2 * j + 2) * 256 >= (knext + 1) * STRIPE):
                        emit_z_stripe(l + 1, knext)
                        knext += 1

        # ---- pooling + MLP (pools released above) ----
        with contextlib.ExitStack() as ctx2:
            ppool = ctx2.enter_context(tc.tile_pool(name="poolp", bufs=1))
            mpool = ctx2.enter_context(
                tc.tile_pool(name="mlpp", bufs=2, space="PSUM"))
            gloc = ppool.tile([H, 2 * gpc], f32, name="gloc")
            ops = [mybir.AluOpType.add, mybir.AluOpType.max]
            for j in range(gpc):
                cols = slice(p.slot * j, p.slot * (j + 1))
                for q in range(2):
                    nc.vector.tensor_reduce(
                        out=gloc[:, gpc * q + j:gpc * q + j + 1],
                        in_=hT[:, cols], axis=mybir.AxisListType.X, op=ops[q])
            nc.sync.dma_start(out=gpool_in[:], in_=gloc[:])
            nc.gpsimd.collective_compute(
                "AllGather", mybir.AluOpType.bypass,
                replica_groups=groups, ins=[gpool_in[:]],
                outs=[gpool_out[:]])
            gall = []
            for q in range(2):
                gt = ppool.tile([H, B], f32, name=f"gall{q}")
                nc.sync.dma_start(
                    out=gt[:].rearrange("f (c j) -> f c j", c=n_cores),
                    in_=gpool_out[:, gpc * q:gpc * (q + 1)].rearrange(
                        "(c f) j -> f c j", c=n_cores))
                gall.append(gt)
            gsum, gmax = gall
            pc_sb = ppool.tile([H, B], f32, name="pc_sb")
            ci_sb = ppool.tile([H, B], f32, name="ci_sb")
            f1w = ppool.tile([H, 3, H], f32, name="f1w")
            f1b = ppool.tile([H, 1], f32, name="f1b")
            f2w = ppool.tile([H, H // 2], f32, name="f2w")
            f2b = ppool.tile([H // 2, 1], f32, name="f2b")
            f3w = ppool.tile([H // 2, 1], f32, name="f3w")
            for name, t in [("padcorr", pc_sb), ("cntinv", ci_sb),
                            ("fc1w", f1w), ("fc1b", f1b), ("fc2w", f2w),
                            ("fc2b", f2b), ("fc3w", f3w)]:
                nc.sync.dma_start(t[:], P[name][:])
            nc.vector.tensor_tensor(out=gsum[:], in0=gsum[:], in1=pc_sb[:],
                                    op=mybir.AluOpType.subtract)
            gmean = ppool.tile([H, B], f32, name="gmean")
            nc.vector.tensor_tensor(out=gmean[:], in0=gsum[:], in1=ci_sb[:],
                                    op=mybir.AluOpType.mult)
            mp1 = mpool.tile([H, B], f32, space="PSUM", name="mp1")
            for i, g in enumerate([gmean, gmax, gsum]):
                nc.tensor.matmul(mp1[:], lhsT=f1w[:, i, :], rhs=g[:],
                                 start=(i == 0), stop=(i == 2))
            m1 = ppool.tile([H, B], f32, name="m1")
            nc.scalar.activation(out=m1[:], in_=mp1[:],
                                 func=mybir.ActivationFunctionType.Relu,
                                 bias=f1b[:, 0:1])
            mp2 = mpool.tile([H // 2, B], f32, space="PSUM", name="mp2")
            nc.tensor.matmul(mp2[:], lhsT=f2w[:], rhs=m1[:], start=True,
                             stop=True)
            m2 = ppool.tile([H // 2, B], f32, name="m2")
            nc.scalar.activation(out=m2[:], in_=mp2[:],
                                 func=mybir.ActivationFunctionType.Relu,
                                 bias=f2b[:, 0:1])
            mp3 = mpool.tile([1, B], f32, space="PSUM", name="mp3")
            nc.tensor.matmul(mp3[:], lhsT=f3w[:], rhs=m2[:], start=True,
                             stop=True)
            ob = ppool.tile([1, B], f32, name="ob")
            nc.vector.tensor_scalar_add(ob[:], mp3[:], float(fc3b_val))
            nc.sync.dma_start(out=out_t[:], in_=ob[:])

    nc.compile()
    return nc


# ============================== entry point ================================

def run(inputs, N, B, n_cores=8, slot=512, L=4, sim=False, linearize=False):
    """Full kernel: plan, build, execute, return [B,1] output."""
    p = build_plan(inputs["edge_index"], inputs["batch"], N, B,
                   n_cores=n_cores, slot=slot, L=L)
    in_maps, fc3b = prepare_inputs(
        p, inputs["x"], inputs["conv_ws"], inputs["conv_bs"],
        inputs["bn_gamma"], inputs["bn_beta"], inputs["bn_mean"],
        inputs["bn_var"], inputs["fc1_w"], inputs["fc1_b"], inputs["fc2_w"],
        inputs["fc2_b"], inputs["fc3_w"], inputs["fc3_b"])
    nc = build_nc(p, fc3b, debug=sim, linearize=linearize)
    if sim:
        from concourse.bass_interp import MultiCoreSim
        ms = MultiCoreSim(nc, num_cores=n_cores)
        for c in range(n_cores):
            for k, v in in_maps[c].items():
                ms.cores[c].tensor(k)[:] = v
        ms.simulate()
        out = np.asarray(ms.cores[0].tensor("out"))
    else:
        from concourse.bass_utils import run_bass_kernel_spmd
        res = run_bass_kernel_spmd(nc, in_maps, list(range(n_cores)))
        out = res.results[0]["out"]
    return out.reshape(B, 1).astype(F32)


# ============================== harness entry ==============================

_N, _B, _L = 100000, 256, 4


def kernel(**inputs):
    """Full-input entry point: shards across 8 NeuronCores internally."""
    inputs = {k: np.asarray(v) for k, v in inputs.items()}
    out = run(inputs, N=_N, B=_B, n_cores=8, slot=512, L=_L, sim=False)
    return out.astype(np.float32)


# revision 15
# speedup vs baseline: 1.0131x; 1.0131x over previous
"""BasicGCN (4x GCNConv+BN+ReLU, mean/max/sum pool, MLP) on 8 TRN2 NeuronCores.

Strategy:
  - Graphs are assigned to cores (32 graphs/core), each graph gets a fixed
    512-column slot; nodes live in their graph's slot => pooling is uniform
    free-axis reduces, no cross-core graphs.
  - Per layer: z = h @ W on PE (feature-major), y = dinv*z, PE-transpose to
    node-major fp16, DMA out, AllGather in 8 stripe pieces (16384-row blocks,
    inside int16 gather reach).
  - Message aggregation: per (256-col call-group cg, src-block b) dma_gather
    of the edge sources (dst-sorted slot list, cross-core-maxed cell sizes,
    16-aligned only => ~1.14x slot inflation), spread round-robin over the
    4 SWDGE queues so descriptor rings drain concurrently (~4x gather rate);
    256-wide one-hot selector columns built on DVE via is_equal(iota, dcol)
    in fp16, PE matmuls (stationary = gathered fp16 chunk, moving = selector
    [k,256]) accumulate scatter-add results in a fp32 [128,256] PSUM region
    per cg.
  - Block order rotated by cg so early call-groups don't head-of-line block
    on late AllGather pieces.
  - Next layer's dense phase (z/transpose/AG stripe k) is emitted right after
    the evictions that produce its hT columns, hiding layer boundaries.
  - Eviction fuses +self_loop, *dinv, BN affine (A,B folded) and ReLU.
  - Pool: 512-col reduces (sum/max) + one merged AllGather + pad correction
    + MLP on PE.
"""
import math
import numpy as np
import ml_dtypes

from concourse import bass, mybir, bacc, tile
from concourse import library_config

F32 = np.float32
FP16 = np.float16
H = 128          # feature dim == partitions
EPS = 1e-5


# ============================== host planning ==============================

class Plan:
    pass


def build_plan(edge_index, batch, N, B, n_cores=8, slot=512, L=4):
    p = Plan()
    assert B % n_cores == 0
    gpc = B // n_cores                 # graphs per core
    P_own = slot * gpc                 # columns per core
    assert P_own % 1024 == 0
    STRIPE = P_own // 8                # rows per AG piece
    assert STRIPE % 128 == 0
    BLOCK = STRIPE * n_cores           # rows per gather block
    assert BLOCK <= 32767
    NCG = P_own // 256                 # call groups (256 cols each)

    src = np.asarray(edge_index[0]).astype(np.int64)
    dst = np.asarray(edge_index[1]).astype(np.int64)
    batch = np.asarray(batch)

    counts = np.bincount(batch, minlength=B).astype(np.int64)
    assert counts.max() <= slot, (counts.max(), slot)
    starts = np.concatenate([[0], np.cumsum(counts)[:-1]])

    # node -> (core, col)
    gr_of = batch.astype(np.int64)
    core_of = gr_of // gpc
    col_of = (gr_of % gpc) * slot + (np.arange(N) - starts[gr_of])
    assert (col_of < P_own).all()

    # gid: global padded row id (stripe-major)
    stripe_of = col_of // STRIPE
    gid = BLOCK * stripe_of + STRIPE * core_of + (col_of % STRIPE)

    deg = np.bincount(dst, minlength=N).astype(np.float64) + 1.0
    dinv = (1.0 / np.sqrt(deg)).astype(F32)

    # per-edge attributes (dst side)
    e_core = core_of[dst]
    e_col = col_of[dst]
    e_cg = e_col // 256
    e_blk = gid[src] // BLOCK
    e_idx = gid[src] % BLOCK

    # cell counts maxed over cores (SPMD: one program, shared static sizes)
    NBLK = 8
    cell = np.zeros((n_cores, NCG, NBLK), np.int64)
    np.add.at(cell, (e_core, e_cg, e_blk), 1)
    ncell = cell.max(axis=0)            # [NCG, NBLK]

    # paired gather calls: call (j, b) spans cgs (2j, 2j+1); the second
    # region starts 32-aligned so matmul pieces stay on legal base partitions
    NPAIR = NCG // 2
    lenA = ncell[0::2, :]                     # [NPAIR, NBLK]
    lenB = ncell[1::2, :]
    obnd = ((lenA + 31) // 32) * 32           # region-B start within call
    call_len = ((obnd + lenB + 15) // 16) * 16
    call_off = np.zeros((NPAIR, NBLK), np.int64)
    call_ccol = np.zeros((NPAIR, NBLK), np.int64)
    off = 0
    ccol = 0
    for j in range(NPAIR):
        for b in range(NBLK):
            call_off[j, b] = off
            call_ccol[j, b] = ccol
            off += int(call_len[j, b])
            ccol += (int(call_len[j, b]) + 127) // 128
    TOT = off
    TOTCOL = ccol
    CCMAX = int(max((int(call_len[j, b]) + 127) // 128
                    for j in range(NPAIR) for b in range(NBLK)))

    # per-core slot data: slots per (cg, b) sorted by dst col
    idx_all = np.zeros((n_cores, TOT), np.int16)
    dcol_all = np.full((n_cores, TOT), -1.0, F32)
    order = np.lexsort((e_col, e_blk, e_cg, e_core))
    eo_core, eo_cg, eo_blk = e_core[order], e_cg[order], e_blk[order]
    eo_idx, eo_col = e_idx[order], e_col[order]
    key = (eo_core * NCG + eo_cg) * NBLK + eo_blk
    uq, st = np.unique(key, return_index=True)
    st = list(st) + [len(key)]
    for u, s0, s1 in zip(uq, st[:-1], st[1:]):
        b = int(u) % NBLK
        cg = (int(u) // NBLK) % NCG
        c = int(u) // (NBLK * NCG)
        n = s1 - s0
        j, r = cg // 2, cg % 2
        pos = int(call_off[j, b]) + (int(obnd[j, b]) if r else 0)
        idx_all[c, pos:pos + n] = eo_idx[s0:s1].astype(np.int16)
        dcol_all[c, pos:pos + n] = (eo_col[s0:s1] % 256).astype(F32)

    # pieces per call (j, b): (ccol, base_part, k, region); bases limited to
    # {0,32,64,96} with per-base K caps (PE sub-quadrant addressing)
    CAP = {0: 128, 32: 32, 64: 64, 96: 32}
    pieces = {}
    for j in range(NPAIR):
        for b in range(NBLK):
            pl = []
            for r, (s0, n) in enumerate([(0, int(lenA[j, b])),
                                         (int(obnd[j, b]), int(lenB[j, b]))]):
                o = int(s0)
                while n > 0:
                    pp = o % 128
                    k = min(CAP[pp], n)
                    pl.append((o // 128, pp, k, r))
                    o += k
                    n -= k
            pieces[(j, b)] = pl

    # block visit order per pair (rotated so pair j starts on AG piece j%8)
    border = {j: [(j + k) % NBLK for k in range(NBLK)] for j in range(NPAIR)}
    # first/last (b, piece i) per cg (psum region) in rotated issue order
    flags = {}
    for cg in range(NCG):
        j, r = cg // 2, cg % 2
        seq = [(b, i) for b in border[j]
               for i, pc in enumerate(pieces[(j, b)]) if pc[3] == r]
        flags[cg] = (seq[0], seq[-1]) if seq else (None, None)

    p.__dict__.update(locals())
    return p


def prepare_inputs(p, x, conv_ws, conv_bs, bn_gamma, bn_beta, bn_mean, bn_var,
                   fc1_w, fc1_b, fc2_w, fc2_b, fc3_w, fc3_b):
    """Build per-core in_maps (list of dicts of np arrays)."""
    n_cores, P_own, N, B, L = p.n_cores, p.P_own, p.N, p.B, p.L
    A = (bn_gamma / np.sqrt(bn_var + EPS)).astype(F32)      # [L,H]
    Bv = ((conv_bs - bn_mean) * A + bn_beta).astype(F32)    # [L,H]
    AB = np.zeros((H, 2 * L), F32)
    for l in range(L):
        AB[:, 2 * l] = A[l]
        AB[:, 2 * l + 1] = Bv[l]
    reluB_last = np.maximum(Bv[L - 1], 0.0)                 # [H]
    padn = (p.slot - p.counts).astype(F32)                  # [B]
    padcorr = np.outer(reluB_last, padn).astype(F32)        # [H, B]
    cntinv = np.tile((1.0 / np.maximum(p.counts, 1.0)).astype(F32), (H, 1))

    Wl = np.zeros((H, L, H), F32)
    for l in range(L):
        Wl[:, l, :] = conv_ws[l]
    fc1 = np.zeros((H, 3, H), F32)
    for k in range(3):
        fc1[:, k, :] = fc1_w[k * H:(k + 1) * H, :]
    iota = np.tile(np.arange(256, dtype=F32)[None, None, :],
                   (128, p.CCMAX, 1)).astype(FP16)
    ident = np.eye(128, dtype=F32)

    in_maps = []
    for c in range(n_cores):
        m = p.core_of == c
        hT0 = np.zeros((H, P_own), F32)
        hT0[:, p.col_of[m]] = np.asarray(x)[m].T
        dr = np.zeros((H, P_own), F32)
        dr[0, p.col_of[m]] = p.dinv[m]
        dr[:, :] = dr[0][None, :]
        # idx wrapped: slot i -> [i%16, i//16], replicated to 128 partitions
        iw = p.idx_all[c].reshape(-1, 16).T            # [16, TOT/16]
        iw = np.tile(iw, (8, 1)).astype(np.int16)      # [128, TOT/16]
        # dcol: slot (128*ccol + pp) -> [pp, ccol]; calls packed by ccol
        dc = np.full((128, p.TOTCOL, 1), -1.0, F32)
        for jj in range(p.NPAIR):
            for b in range(p.NBLK):
                o = int(p.call_off[jj, b])
                ln = int(p.call_len[jj, b])
                cc0 = int(p.call_ccol[jj, b])
                seg = np.full(((ln + 127) // 128) * 128, -1.0, F32)
                seg[:ln] = p.dcol_all[c, o:o + ln]
                dc[:, cc0:cc0 + len(seg) // 128, 0] = seg.reshape(-1, 128).T
        in_maps.append(dict(
            hT0=hT0, dinv_rep=dr, idx=iw, dcol=dc.astype(FP16),
            iota=iota, ident=ident, Wl=Wl, AB=AB,
            padcorr=padcorr, cntinv=cntinv, fc1w=fc1,
            fc1b=np.asarray(fc1_b, F32).reshape(H, 1),
            fc2w=np.asarray(fc2_w, F32),
            fc2b=np.asarray(fc2_b, F32).reshape(H // 2, 1),
            fc3w=np.asarray(fc3_w, F32).reshape(H // 2, 1),
        ))
    return in_maps, float(np.asarray(fc3_b).reshape(-1)[0])


# ============================== device program =============================

def build_nc(p, fc3b_val, debug=False, linearize=False):
    n_cores, P_own, L = p.n_cores, p.P_own, p.L
    STRIPE, BLOCK, NCG, NBLK = p.STRIPE, p.BLOCK, p.NCG, p.NBLK
    B = p.B
    gpc = p.gpc
    CCMAX = p.CCMAX
    f32, fp16, i16 = mybir.dt.float32, mybir.dt.float16, mybir.dt.int16

    nc = bacc.Bacc("TRN2", target_bir_lowering=False, debug=debug,
                   num_devices=n_cores, num_swdge_queues=4)
    groups = [list(range(n_cores))]

    # dram parameters
    P = {}
    P["hT0"] = nc.dram_tensor("hT0", [H, P_own], f32, kind="ExternalInput")
    P["dinv_rep"] = nc.dram_tensor("dinv_rep", [H, P_own], f32,
                                   kind="ExternalInput")
    P["idx"] = nc.dram_tensor("idx", [128, p.TOT // 16], i16,
                              kind="ExternalInput")
    P["dcol"] = nc.dram_tensor("dcol", [128, p.TOTCOL, 1], fp16,
                               kind="ExternalInput")
    P["iota"] = nc.dram_tensor("iota", [128, CCMAX, 256], fp16,
                               kind="ExternalInput")
    P["ident"] = nc.dram_tensor("ident", [128, 128], f32,
                                kind="ExternalInput")
    P["Wl"] = nc.dram_tensor("Wl", [H, L, H], f32, kind="ExternalInput")
    P["AB"] = nc.dram_tensor("AB", [H, 2 * L], f32, kind="ExternalInput")
    P["padcorr"] = nc.dram_tensor("padcorr", [H, B], f32,
                                  kind="ExternalInput")
    P["cntinv"] = nc.dram_tensor("cntinv", [H, B], f32, kind="ExternalInput")
    P["fc1w"] = nc.dram_tensor("fc1w", [H, 3, H], f32, kind="ExternalInput")
    P["fc1b"] = nc.dram_tensor("fc1b", [H, 1], f32, kind="ExternalInput")
    P["fc2w"] = nc.dram_tensor("fc2w", [H, H // 2], f32, kind="ExternalInput")
    P["fc2b"] = nc.dram_tensor("fc2b", [H // 2, 1], f32, kind="ExternalInput")
    P["fc3w"] = nc.dram_tensor("fc3w", [H // 2, 1], f32, kind="ExternalInput")
    out_t = nc.dram_tensor("out", [1, B], f32, kind="ExternalOutput")

    # dram internals
    ynode_d = nc.dram_tensor("ynode_d", [P_own, H], fp16)
    yfull = [nc.dram_tensor(f"yfull{v}", [BLOCK * 8, H], fp16,
                            addr_space="Shared") for v in range(2)]
    gpool_in = nc.dram_tensor("gpool_in", [H, 2 * gpc], f32)
    gpool_out = nc.dram_tensor("gpool_out", [H * n_cores, 2 * gpc], f32,
                               addr_space="Shared")

    L16MAX = int(p.call_len.max() // 16)
    NT = STRIPE // 128           # transposes per stripe

    import contextlib
    with tile.TileContext(nc, linearize=linearize) as tc, \
            contextlib.ExitStack() as octx:
        nc.gpsimd.load_library(library_config.mlp)
        cpool = octx.enter_context(tc.tile_pool(name="consts", bufs=1))
        with contextlib.ExitStack() as ctx:
            hT = cpool.tile([H, P_own], f32)
            dinv = cpool.tile([H, P_own], f32)
            dcol_sb = cpool.tile([128, p.TOTCOL, 1], fp16)
            iota = cpool.tile([128, CCMAX, 256], fp16)
            ident = cpool.tile([128, 128], f32)
            W_sb = cpool.tile([H, L, H], f32)
            AB_sb = cpool.tile([H, 2 * L], f32)
            for name, t in [("hT0", hT), ("dinv_rep", dinv),
                            ("dcol", dcol_sb), ("iota", iota),
                            ("ident", ident), ("Wl", W_sb), ("AB", AB_sb)]:
                nc.sync.dma_start(t[:], P[name][:])
            gloc = cpool.tile([H, 2 * gpc], f32, name="gloc")
            pops = [mybir.AluOpType.add, mybir.AluOpType.max]

            ynpool = ctx.enter_context(tc.tile_pool(name="ynp", bufs=2))
            stpool = ctx.enter_context(tc.tile_pool(name="stp", bufs=8))
            sgpool = ctx.enter_context(tc.tile_pool(name="sgp", bufs=6))
            ixpool = ctx.enter_context(tc.tile_pool(name="ixp", bufs=8))
            evpool = ctx.enter_context(tc.tile_pool(name="evp", bufs=4))
            zpool = ctx.enter_context(
                tc.tile_pool(name="zp", bufs=2, space="PSUM"))
            tpool = ctx.enter_context(
                tc.tile_pool(name="tp", bufs=2, space="PSUM"))
            apool = ctx.enter_context(
                tc.tile_pool(name="ap", bufs=4, space="PSUM"))

            def emit_z_stripe(l, k):
                """z = W^T h, y = dinv*z (in place), transpose, DMA, AG."""
                yf = yfull[l % 2]
                base = STRIPE * k
                o = 0
                while o < STRIPE:
                    w = min(512, STRIPE - o)
                    zp = zpool.tile([128, 512], f32, space="PSUM",
                                    tag="zp", name="zpt")
                    cols = slice(base + o, base + o + w)
                    nc.tensor.matmul(zp[:, :w], lhsT=W_sb[:, l, :],
                                     rhs=hT[:, cols], start=True, stop=True)
                    nc.vector.tensor_tensor(
                        out=hT[:, cols], in0=zp[:, :w],
                        in1=dinv[:, cols], op=mybir.AluOpType.mult)
                    o += w
                yn_sb = ynpool.tile([128, NT, 128], fp16, tag="yn",
                                    name="ynt")
                for t in range(NT):
                    tp = tpool.tile([128, 128], f32, space="PSUM",
                                    tag="tp", name="tpt")
                    cols = slice(base + 128 * t, base + 128 * (t + 1))
                    nc.tensor.transpose(tp[:], hT[:, cols], ident[:])
                    nc.scalar.activation(
                        out=yn_sb[:, t, :], in_=tp[:],
                        func=mybir.ActivationFunctionType.Copy)
                dview = ynode_d[base:base + STRIPE, :].rearrange(
                    "(t q) f -> q t f", q=128)
                nc.sync.dma_start(out=dview, in_=yn_sb[:])
                nc.gpsimd.collective_compute(
                    "AllGather", mybir.AluOpType.bypass,
                    replica_groups=groups,
                    ins=[ynode_d[base:base + STRIPE, :]],
                    outs=[yf[BLOCK * k:BLOCK * (k + 1), :]])

            qctr = [0]

            def emit_agg_pair(l, j):
                """Gather+scatter-add 512 dst cols (cgs 2j,2j+1), evict."""
                yf = yfull[l % 2]
                ps = [apool.tile([128, 256], f32, space="PSUM",
                                 tag="agg", name="aggt") for _ in range(2)]
                fl = [p.flags[2 * j], p.flags[2 * j + 1]]
                for b in p.border[j]:
                    ln = int(p.call_len[j, b])
                    if ln == 0:
                        continue
                    cc0 = int(p.call_ccol[j, b])
                    ncc = (ln + 127) // 128
                    o16 = int(p.call_off[j, b]) // 16
                    ixt = ixpool.tile([128, L16MAX], i16, tag="ix",
                                      name="ixt")
                    nc.sync.dma_start(ixt[:, :ln // 16],
                                      P["idx"][:, o16:o16 + ln // 16])
                    stag = stpool.tile([128, CCMAX, 128], fp16, tag="st",
                                       name="stagt")
                    # HW limit: <=1024 idxs (64 descs/engine) per call
                    for o in range(0, ln, 1024):
                        sub = min(1024, ln - o)
                        nc.gpsimd.dma_gather(
                            stag[:, o // 128:(o + sub + 127) // 128, :],
                            yf[BLOCK * b:BLOCK * (b + 1), :],
                            ixt[:, o // 16:(o + sub) // 16],
                            sub, sub, 128, single_packet=True,
                            queue_num=qctr[0] % 4)
                        qctr[0] += 1
                    st = sgpool.tile([128, CCMAX, 256], fp16, tag="sg",
                                     name="sgt")
                    nc.vector.tensor_tensor(
                        out=st[:, :ncc, :], in0=iota[:, :ncc, :],
                        in1=dcol_sb[:, cc0:cc0 + ncc, :]
                            .to_broadcast([128, ncc, 256]),
                        op=mybir.AluOpType.is_equal)
                    for i, (cc, pp, kk, r) in enumerate(p.pieces[(j, b)]):
                        first, last = fl[r]
                        nc.tensor.matmul(
                            ps[r][:], lhsT=stag[pp:pp + kk, cc, :],
                            rhs=st[pp:pp + kk, cc, :],
                            start=(first == (b, i)), stop=(last == (b, i)),
                            tile_position=((96, 0) if pp == 96 else None))
                for r in range(2):
                    cg = 2 * j + r
                    first = fl[r][0]
                    cols = slice(256 * cg, 256 * (cg + 1))
                    t2 = evpool.tile([128, 256], f32, tag="ev2", name="ev2t")
                    if first is not None:
                        t1 = evpool.tile([128, 256], f32, tag="ev1",
                                         name="ev1t")
                        nc.vector.tensor_tensor(
                            out=t1[:], in0=ps[r][:], in1=hT[:, cols],
                            op=mybir.AluOpType.add)
                        nc.vector.tensor_tensor(
                            out=t2[:], in0=t1[:], in1=dinv[:, cols],
                            op=mybir.AluOpType.mult)
                    else:
                        nc.vector.tensor_tensor(
                            out=t2[:], in0=hT[:, cols], in1=dinv[:, cols],
                            op=mybir.AluOpType.mult)
                    nc.scalar.activation(
                        out=hT[:, cols], in_=t2[:],
                        func=mybir.ActivationFunctionType.Relu,
                        scale=AB_sb[:, 2 * l:2 * l + 1],
                        bias=AB_sb[:, 2 * l + 1:2 * l + 2])

            # layer 0 dense phase up front; later layers interleave with the
            # previous layer's aggregation (stripe k ready after cg 4k+3)
            for k in range(8):
                emit_z_stripe(0, k)
            for l in range(L):
                knext = 0            # next stripe of layer l+1 to emit
                for j in range(p.NPAIR):
                    emit_agg_pair(l, j)
                    if l == L - 1 and p.slot * (j + 1) <= P_own:
                        # pair j's 512 cols == graph j: pool it immediately
                        pcols = slice(p.slot * j, p.slot * (j + 1))
                        for q in range(2):
                            nc.vector.tensor_reduce(
                                out=gloc[:, gpc * q + j:gpc * q + j + 1],
                                in_=hT[:, pcols], axis=mybir.AxisListType.X,
                                op=pops[q])
                    while (l < L - 1 and knext < 8
                           and (2 * j + 2) * 256 >= (knext + 1) * STRIPE):
                        emit_z_stripe(l + 1, knext)
                        knext += 1

        # ---- pooling + MLP (pools released above) ----
        with contextlib.ExitStack() as ctx2:
            ppool = ctx2.enter_context(tc.tile_pool(name="poolp", bufs=1))
            mpool = ctx2.enter_context(
                tc.tile_pool(name="mlpp", bufs=2, space="PSUM"))
            nc.sync.dma_start(out=gpool_in[:], in_=gloc[:])
            nc.gpsimd.collective_compute(
                "AllGather", mybir.AluOpType.bypass,
                replica_groups=groups, ins=[gpool_in[:]],
                outs=[gpool_out[:]])
            gall = []
            for q in range(2):
                gt = ppool.tile([H, B], f32, name=f"gall{q}")
                nc.sync.dma_start(
                    out=gt[:].rearrange("f (c j) -> f c j", c=n_cores),
                    in_=gpool_out[:, gpc * q:gpc * (q + 1)].rearrange(
                        "(c f) j -> f c j", c=n_cores))
                gall.append(gt)
            gsum, gmax = gall
            pc_sb = ppool.tile([H, B], f32, name="pc_sb")
            ci_sb = ppool.tile([H, B], f32, name="ci_sb")
            f1w = ppool.tile([H, 3, H], f32, name="f1w")
            f1b = ppool.tile([H, 1], f32, name="f1b")
            f2w = ppool.tile([H, H // 2], f32, name="f2w")
            f2b = ppool.tile([H // 2, 1], f32, name="f2b")
            f3w = ppool.tile([H // 2, 1], f32, name="f3w")
            for name, t in [("padcorr", pc_sb), ("cntinv", ci_sb),
                            ("fc1w", f1w), ("fc1b", f1b), ("fc2w", f2w),
                            ("fc2b", f2b), ("fc3w", f3w)]:
                nc.sync.dma_start(t[:], P[name][:])
            nc.vector.tensor_tensor(out=gsum[:], in0=gsum[:], in1=pc_sb[:],
                                    op=mybir.AluOpType.subtract)
            gmean = ppool.tile([H, B], f32, name="gmean")
            nc.vector.tensor_tensor(out=gmean[:], in0=gsum[:], in1=ci_sb[:],
                                    op=mybir.AluOpType.mult)
            mp1 = mpool.tile([H, B], f32, space="PSUM", name="mp1")
            for i, g in enumerate([gmean, gmax, gsum]):
                nc.tensor.matmul(mp1[:], lhsT=f1w[:, i, :], rhs=g[:],
                                 start=(i == 0), stop=(i == 2))
            m1 = ppool.tile([H, B], f32, name="m1")
            nc.scalar.activation(out=m1[:], in_=mp1[:],
                                 func=mybir.ActivationFunctionType.Relu,
                                 bias=f1b[:, 0:1])
            mp2 = mpool.tile([H // 2, B], f32, space="PSUM", name="mp2")
            nc.tensor.matmul(mp2[:], lhsT=f2w[:], rhs=m1[:], start=True,
                             stop=True)
            m2 = ppool.tile([H // 2, B], f32, name="m2")
            nc.scalar.activation(out=m2[:], in_=mp2[:],
                                 func=mybir.ActivationFunctionType.Relu,
                                 bias=f2b[:, 0:1])
            mp3 = mpool.tile([1, B], f32, space="PSUM", name="mp3")
            nc.tensor.matmul(mp3[:], lhsT=f3w[:], rhs=m2[:], start=True,
                             stop=True)
            ob = ppool.tile([1, B], f32, name="ob")
            nc.vector.tensor_scalar_add(ob[:], mp3[:], float(fc3b_val))
            nc.sync.dma_start(out=out_t[:], in_=ob[:])

    nc.compile()
    return nc


# ============================== entry point ================================

def run(inputs, N, B, n_cores=8, slot=512, L=4, sim=False, linearize=False):
    """Full kernel: plan, build, execute, return [B,1] output."""
    p = build_plan(inputs["edge_index"], inputs["batch"], N, B,
                   n_cores=n_cores, slot=slot, L=L)
    in_maps, fc3b = prepare_inputs(
        p, inputs["x"], inputs["conv_ws"], inputs["conv_bs"],
        inputs["bn_gamma"], inputs["bn_beta"], inputs["bn_mean"],
        inputs["bn_var"], inputs["fc1_w"], inputs["fc1_b"], inputs["fc2_w"],
        inputs["fc2_b"], inputs["fc3_w"], inputs["fc3_b"])
    nc = build_nc(p, fc3b, debug=sim, linearize=linearize)
    if sim:
        from concourse.bass_interp import MultiCoreSim
        ms = MultiCoreSim(nc, num_cores=n_cores)
        for c in range(n_cores):
            for k, v in in_maps[c].items():
                ms.cores[c].tensor(k)[:] = v
        ms.simulate()
        out = np.asarray(ms.cores[0].tensor("out"))
    else:
        from concourse.bass_utils import run_bass_kernel_spmd
        res = run_bass_kernel_spmd(nc, in_maps, list(range(n_cores)))
        out = res.results[0]["out"]
    return out.reshape(B, 1).astype(F32)


# ============================== harness entry ==============================

_N, _B, _L = 100000, 256, 4


def kernel(**inputs):
    """Full-input entry point: shards across 8 NeuronCores internally."""
    inputs = {k: np.asarray(v) for k, v in inputs.items()}
    out = run(inputs, N=_N, B=_B, n_cores=8, slot=512, L=_L, sim=False)
    return out.astype(np.float32)


# revision 18
# speedup vs baseline: 1.0795x; 1.0655x over previous
"""BasicGCN (4x GCNConv+BN+ReLU, mean/max/sum pool, MLP) on 8 TRN2 NeuronCores.

Strategy:
  - Graphs are assigned to cores (32 graphs/core), each graph gets a fixed
    512-column slot; nodes live in their graph's slot => pooling is uniform
    free-axis reduces, no cross-core graphs.
  - Per layer: z = h @ W on PE (feature-major), y = dinv*z, PE-transpose to
    node-major fp16, DMA out, AllGather in 8 stripe pieces (16384-row blocks,
    inside int16 gather reach).
  - Message aggregation: per (256-col call-group cg, src-block b) dma_gather
    of the edge sources (dst-sorted slot list, cross-core-maxed cell sizes,
    16-aligned only => ~1.14x slot inflation), spread round-robin over the
    4 SWDGE queues so descriptor rings drain concurrently (~4x gather rate);
    256-wide one-hot selector columns built on DVE via is_equal(iota, dcol)
    in fp16, PE matmuls (stationary = gathered fp16 chunk, moving = selector
    [k,256]) accumulate scatter-add results in a fp32 [128,256] PSUM region
    per cg.
  - Block order rotated by cg so early call-groups don't head-of-line block
    on late AllGather pieces.
  - Next layer's dense phase (z/transpose/AG stripe k) is emitted right after
    the evictions that produce its hT columns, hiding layer boundaries.
  - Eviction fuses +self_loop, *dinv, BN affine (A,B folded) and ReLU.
  - Pool: 512-col reduces (sum/max) + one merged AllGather + pad correction
    + MLP on PE.
"""
import math
import numpy as np
import ml_dtypes

from concourse import bass, mybir, bacc, tile
from concourse import library_config

F32 = np.float32
FP16 = np.float16
H = 128          # feature dim == partitions
EPS = 1e-5


# ============================== host planning ==============================

class Plan:
    pass


def build_plan(edge_index, batch, N, B, n_cores=8, slot=512, L=4):
    p = Plan()
    assert B % n_cores == 0
    gpc = B // n_cores                 # graphs per core
    P_own = slot * gpc                 # columns per core
    assert P_own % 1024 == 0
    STRIPE = P_own // 8                # rows per AG piece
    assert STRIPE % 128 == 0
    BLOCK = STRIPE * n_cores           # rows per gather block
    assert BLOCK <= 32767
    NCG = P_own // 256                 # call groups (256 cols each)

    src = np.asarray(edge_index[0]).astype(np.int64)
    dst = np.asarray(edge_index[1]).astype(np.int64)
    batch = np.asarray(batch)

    counts = np.bincount(batch, minlength=B).astype(np.int64)
    assert counts.max() <= slot, (counts.max(), slot)
    starts = np.concatenate([[0], np.cumsum(counts)[:-1]])

    # node -> (core, col)
    gr_of = batch.astype(np.int64)
    core_of = gr_of // gpc
    col_of = (gr_of % gpc) * slot + (np.arange(N) - starts[gr_of])
    assert (col_of < P_own).all()

    # gid: global padded row id (stripe-major)
    stripe_of = col_of // STRIPE
    gid = BLOCK * stripe_of + STRIPE * core_of + (col_of % STRIPE)

    deg = np.bincount(dst, minlength=N).astype(np.float64) + 1.0
    dinv = (1.0 / np.sqrt(deg)).astype(F32)

    # per-edge attributes (dst side)
    e_core = core_of[dst]
    e_col = col_of[dst]
    e_cg = e_col // 256
    e_blk = gid[src] // BLOCK
    e_idx = gid[src] % BLOCK

    # cell counts maxed over cores (SPMD: one program, shared static sizes)
    NBLK = 8
    cell = np.zeros((n_cores, NCG, NBLK), np.int64)
    np.add.at(cell, (e_core, e_cg, e_blk), 1)
    ncell = cell.max(axis=0)            # [NCG, NBLK]

    # paired gather calls: call (j, b) spans cgs (2j, 2j+1); the second
    # region starts 32-aligned so matmul pieces stay on legal base partitions
    NPAIR = NCG // 2
    lenA = ncell[0::2, :]                     # [NPAIR, NBLK]
    lenB = ncell[1::2, :]
    obnd = ((lenA + 31) // 32) * 32           # region-B start within call
    call_len = ((obnd + lenB + 15) // 16) * 16
    call_off = np.zeros((NPAIR, NBLK), np.int64)
    call_ccol = np.zeros((NPAIR, NBLK), np.int64)
    off = 0
    ccol = 0
    for j in range(NPAIR):
        for b in range(NBLK):
            call_off[j, b] = off
            call_ccol[j, b] = ccol
            off += int(call_len[j, b])
            ccol += (int(call_len[j, b]) + 127) // 128
    TOT = off
    TOTCOL = ccol
    CCMAX = int(max((int(call_len[j, b]) + 127) // 128
                    for j in range(NPAIR) for b in range(NBLK)))

    # per-core slot data: slots per (cg, b) sorted by dst col
    idx_all = np.zeros((n_cores, TOT), np.int16)
    dcol_all = np.full((n_cores, TOT), -1.0, F32)
    order = np.lexsort((e_col, e_blk, e_cg, e_core))
    eo_core, eo_cg, eo_blk = e_core[order], e_cg[order], e_blk[order]
    eo_idx, eo_col = e_idx[order], e_col[order]
    key = (eo_core * NCG + eo_cg) * NBLK + eo_blk
    uq, st = np.unique(key, return_index=True)
    st = list(st) + [len(key)]
    for u, s0, s1 in zip(uq, st[:-1], st[1:]):
        b = int(u) % NBLK
        cg = (int(u) // NBLK) % NCG
        c = int(u) // (NBLK * NCG)
        n = s1 - s0
        j, r = cg // 2, cg % 2
        pos = int(call_off[j, b]) + (int(obnd[j, b]) if r else 0)
        idx_all[c, pos:pos + n] = eo_idx[s0:s1].astype(np.int16)
        dcol_all[c, pos:pos + n] = (eo_col[s0:s1] % 256).astype(F32)

    # pieces per call (j, b): (ccol, base_part, k, region); bases limited to
    # {0,32,64,96} with per-base K caps (PE sub-quadrant addressing)
    CAP = {0: 128, 32: 32, 64: 64, 96: 32}
    pieces = {}
    for j in range(NPAIR):
        for b in range(NBLK):
            pl = []
            for r, (s0, n) in enumerate([(0, int(lenA[j, b])),
                                         (int(obnd[j, b]), int(lenB[j, b]))]):
                o = int(s0)
                while n > 0:
                    pp = o % 128
                    k = min(CAP[pp], n)
                    pl.append((o // 128, pp, k, r))
                    o += k
                    n -= k
            pieces[(j, b)] = pl

    # block visit order per pair (rotated so pair j starts on AG piece j%8)
    border = {j: [(j + k) % NBLK for k in range(NBLK)] for j in range(NPAIR)}
    # first/last (b, piece i) per cg (psum region) in rotated issue order
    flags = {}
    for cg in range(NCG):
        j, r = cg // 2, cg % 2
        seq = [(b, i) for b in border[j]
               for i, pc in enumerate(pieces[(j, b)]) if pc[3] == r]
        flags[cg] = (seq[0], seq[-1]) if seq else (None, None)

    p.__dict__.update(locals())
    return p


def prepare_inputs(p, x, conv_ws, conv_bs, bn_gamma, bn_beta, bn_mean, bn_var,
                   fc1_w, fc1_b, fc2_w, fc2_b, fc3_w, fc3_b):
    """Build per-core in_maps (list of dicts of np arrays)."""
    n_cores, P_own, N, B, L = p.n_cores, p.P_own, p.N, p.B, p.L
    A = (bn_gamma / np.sqrt(bn_var + EPS)).astype(F32)      # [L,H]
    Bv = ((conv_bs - bn_mean) * A + bn_beta).astype(F32)    # [L,H]
    AB = np.zeros((H, 2 * L), F32)
    for l in range(L):
        AB[:, 2 * l] = A[l]
        AB[:, 2 * l + 1] = Bv[l]
    reluB_last = np.maximum(Bv[L - 1], 0.0)                 # [H]
    padn = (p.slot - p.counts).astype(F32)                  # [B]
    padcorr = np.outer(reluB_last, padn).astype(F32)        # [H, B]
    cntinv = np.tile((1.0 / np.maximum(p.counts, 1.0)).astype(F32), (H, 1))

    Wl = np.zeros((H, L, H), F32)
    for l in range(L):
        Wl[:, l, :] = conv_ws[l]
    fc1 = np.zeros((H, 3, H), F32)
    for k in range(3):
        fc1[:, k, :] = fc1_w[k * H:(k + 1) * H, :]
    iota = np.tile(np.arange(256, dtype=F32)[None, None, :],
                   (128, p.CCMAX, 1)).astype(FP16)
    ident = np.eye(128, dtype=F32)

    in_maps = []
    for c in range(n_cores):
        m = p.core_of == c
        hT0 = np.zeros((H, P_own), F32)
        hT0[:, p.col_of[m]] = np.asarray(x)[m].T
        dr = np.zeros((H, P_own), F32)
        dr[0, p.col_of[m]] = p.dinv[m]
        dr[:, :] = dr[0][None, :]
        # idx wrapped: slot i -> [i%16, i//16], replicated to 128 partitions
        iw = p.idx_all[c].reshape(-1, 16).T            # [16, TOT/16]
        iw = np.tile(iw, (8, 1)).astype(np.int16)      # [128, TOT/16]
        # dcol: slot (128*ccol + pp) -> [pp, ccol]; calls packed by ccol
        dc = np.full((128, p.TOTCOL, 1), -1.0, F32)
        for jj in range(p.NPAIR):
            for b in range(p.NBLK):
                o = int(p.call_off[jj, b])
                ln = int(p.call_len[jj, b])
                cc0 = int(p.call_ccol[jj, b])
                seg = np.full(((ln + 127) // 128) * 128, -1.0, F32)
                seg[:ln] = p.dcol_all[c, o:o + ln]
                dc[:, cc0:cc0 + len(seg) // 128, 0] = seg.reshape(-1, 128).T
        in_maps.append(dict(
            hT0=hT0, dinv_rep=dr, idx=iw, dcol=dc.astype(FP16),
            iota=iota, ident=ident, Wl=Wl, AB=AB,
            padcorr=padcorr, cntinv=cntinv, fc1w=fc1,
            fc1b=np.asarray(fc1_b, F32).reshape(H, 1),
            fc2w=np.asarray(fc2_w, F32),
            fc2b=np.asarray(fc2_b, F32).reshape(H // 2, 1),
            fc3w=np.asarray(fc3_w, F32).reshape(H // 2, 1),
        ))
    return in_maps, float(np.asarray(fc3_b).reshape(-1)[0])


# ============================== device program =============================

def build_nc(p, fc3b_val, debug=False, linearize=False):
    n_cores, P_own, L = p.n_cores, p.P_own, p.L
    STRIPE, BLOCK, NCG, NBLK = p.STRIPE, p.BLOCK, p.NCG, p.NBLK
    B = p.B
    gpc = p.gpc
    CCMAX = p.CCMAX
    f32, fp16, i16 = mybir.dt.float32, mybir.dt.float16, mybir.dt.int16

    nc = bacc.Bacc("TRN2", target_bir_lowering=False, debug=debug,
                   num_devices=n_cores, num_swdge_queues=4)
    groups = [list(range(n_cores))]

    # dram parameters
    P = {}
    P["hT0"] = nc.dram_tensor("hT0", [H, P_own], f32, kind="ExternalInput")
    P["dinv_rep"] = nc.dram_tensor("dinv_rep", [H, P_own], f32,
                                   kind="ExternalInput")
    P["idx"] = nc.dram_tensor("idx", [128, p.TOT // 16], i16,
                              kind="ExternalInput")
    P["dcol"] = nc.dram_tensor("dcol", [128, p.TOTCOL, 1], fp16,
                               kind="ExternalInput")
    P["iota"] = nc.dram_tensor("iota", [128, CCMAX, 256], fp16,
                               kind="ExternalInput")
    P["ident"] = nc.dram_tensor("ident", [128, 128], f32,
                                kind="ExternalInput")
    P["Wl"] = nc.dram_tensor("Wl", [H, L, H], f32, kind="ExternalInput")
    P["AB"] = nc.dram_tensor("AB", [H, 2 * L], f32, kind="ExternalInput")
    P["padcorr"] = nc.dram_tensor("padcorr", [H, B], f32,
                                  kind="ExternalInput")
    P["cntinv"] = nc.dram_tensor("cntinv", [H, B], f32, kind="ExternalInput")
    P["fc1w"] = nc.dram_tensor("fc1w", [H, 3, H], f32, kind="ExternalInput")
    P["fc1b"] = nc.dram_tensor("fc1b", [H, 1], f32, kind="ExternalInput")
    P["fc2w"] = nc.dram_tensor("fc2w", [H, H // 2], f32, kind="ExternalInput")
    P["fc2b"] = nc.dram_tensor("fc2b", [H // 2, 1], f32, kind="ExternalInput")
    P["fc3w"] = nc.dram_tensor("fc3w", [H // 2, 1], f32, kind="ExternalInput")
    out_t = nc.dram_tensor("out", [1, B], f32, kind="ExternalOutput")

    # dram internals
    ynode_d = nc.dram_tensor("ynode_d", [P_own, H], fp16)
    yfull = [nc.dram_tensor(f"yfull{v}", [BLOCK * 8, H], fp16,
                            addr_space="Shared") for v in range(2)]
    gpool_in = nc.dram_tensor("gpool_in", [H, 2 * gpc], f32)
    gpool_out = nc.dram_tensor("gpool_out", [H * n_cores, 2 * gpc], f32,
                               addr_space="Shared")

    L16MAX = int(p.call_len.max() // 16)
    PAIR16MAX = int(max(
        (int(p.call_off[j, NBLK - 1] + p.call_len[j, NBLK - 1]
             - p.call_off[j, 0])) // 16 for j in range(p.NPAIR)))
    NT = STRIPE // 128           # transposes per stripe

    import contextlib
    with tile.TileContext(nc, linearize=linearize) as tc, \
            contextlib.ExitStack() as octx:
        nc.gpsimd.load_library(library_config.mlp)
        cpool = octx.enter_context(tc.tile_pool(name="consts", bufs=1))
        with contextlib.ExitStack() as ctx:
            hT = cpool.tile([H, P_own], f32)
            dinv = cpool.tile([H, P_own], f32)
            dcol_sb = cpool.tile([128, p.TOTCOL, 1], fp16)
            iota = cpool.tile([128, CCMAX, 256], fp16)
            ident = cpool.tile([128, 128], f32)
            W_sb = cpool.tile([H, L, H], f32)
            AB_sb = cpool.tile([H, 2 * L], f32)
            for name, t in [("hT0", hT), ("dinv_rep", dinv),
                            ("dcol", dcol_sb), ("iota", iota),
                            ("ident", ident), ("Wl", W_sb), ("AB", AB_sb)]:
                nc.sync.dma_start(t[:], P[name][:])
            gloc = cpool.tile([H, 2 * gpc], f32, name="gloc")
            pops = [mybir.AluOpType.add, mybir.AluOpType.max]

            ynpool = ctx.enter_context(tc.tile_pool(name="ynp", bufs=2))
            stpool = ctx.enter_context(tc.tile_pool(name="stp", bufs=8))
            sgpool = ctx.enter_context(tc.tile_pool(name="sgp", bufs=6))
            ixpool = ctx.enter_context(tc.tile_pool(name="ixp", bufs=3))
            evpool = ctx.enter_context(tc.tile_pool(name="evp", bufs=4))
            zpool = ctx.enter_context(
                tc.tile_pool(name="zp", bufs=1, space="PSUM"))
            tpool = ctx.enter_context(
                tc.tile_pool(name="tp", bufs=2, space="PSUM"))
            apool = ctx.enter_context(
                tc.tile_pool(name="ap", bufs=5, space="PSUM"))

            def emit_z_stripe(l, k):
                """z = W^T h, y = dinv*z (in place), transpose, DMA, AG."""
                yf = yfull[l % 2]
                base = STRIPE * k
                o = 0
                while o < STRIPE:
                    w = min(512, STRIPE - o)
                    zp = zpool.tile([128, 512], f32, space="PSUM",
                                    tag="zp", name="zpt")
                    cols = slice(base + o, base + o + w)
                    nc.tensor.matmul(zp[:, :w], lhsT=W_sb[:, l, :],
                                     rhs=hT[:, cols], start=True, stop=True)
                    nc.vector.tensor_tensor(
                        out=hT[:, cols], in0=zp[:, :w],
                        in1=dinv[:, cols], op=mybir.AluOpType.mult)
                    o += w
                yn_sb = ynpool.tile([128, NT, 128], fp16, tag="yn",
                                    name="ynt")
                for t in range(NT):
                    tp = tpool.tile([128, 128], f32, space="PSUM",
                                    tag="tp", name="tpt")
                    cols = slice(base + 128 * t, base + 128 * (t + 1))
                    nc.tensor.transpose(tp[:], hT[:, cols], ident[:])
                    nc.scalar.activation(
                        out=yn_sb[:, t, :], in_=tp[:],
                        func=mybir.ActivationFunctionType.Copy)
                dview = ynode_d[base:base + STRIPE, :].rearrange(
                    "(t q) f -> q t f", q=128)
                nc.sync.dma_start(out=dview, in_=yn_sb[:])
                nc.gpsimd.collective_compute(
                    "AllGather", mybir.AluOpType.bypass,
                    replica_groups=groups,
                    ins=[ynode_d[base:base + STRIPE, :]],
                    outs=[yf[BLOCK * k:BLOCK * (k + 1), :]])

            qctr = [0]

            def emit_agg_pair(l, j):
                """Gather+scatter-add 512 dst cols (cgs 2j,2j+1), evict."""
                yf = yfull[l % 2]
                ps = [apool.tile([128, 256], f32, space="PSUM",
                                 tag="agg", name="aggt") for _ in range(2)]
                fl = [p.flags[2 * j], p.flags[2 * j + 1]]
                p16 = int(p.call_off[j, 0]) // 16
                pln16 = int(p.call_off[j, p.NBLK - 1]
                            + p.call_len[j, p.NBLK - 1]
                            - p.call_off[j, 0]) // 16
                ixt = ixpool.tile([128, PAIR16MAX], i16, tag="ix",
                                  name="ixt")
                if pln16 > 0:
                    nc.sync.dma_start(ixt[:, :pln16],
                                      P["idx"][:, p16:p16 + pln16])
                for b in p.border[j]:
                    ln = int(p.call_len[j, b])
                    if ln == 0:
                        continue
                    cc0 = int(p.call_ccol[j, b])
                    ncc = (ln + 127) // 128
                    b16 = int(p.call_off[j, b]) // 16 - p16
                    stag = stpool.tile([128, CCMAX, 128], fp16, tag="st",
                                       name="stagt")
                    # HW limit: <=1024 idxs (64 descs/engine) per call
                    for o in range(0, ln, 1024):
                        sub = min(1024, ln - o)
                        nc.gpsimd.dma_gather(
                            stag[:, o // 128:(o + sub + 127) // 128, :],
                            yf[BLOCK * b:BLOCK * (b + 1), :],
                            ixt[:, b16 + o // 16:b16 + (o + sub) // 16],
                            sub, sub, 128, single_packet=True,
                            queue_num=qctr[0] % 4)
                        qctr[0] += 1
                    st = sgpool.tile([128, CCMAX, 256], fp16, tag="sg",
                                     name="sgt")
                    nc.vector.tensor_tensor(
                        out=st[:, :ncc, :], in0=iota[:, :ncc, :],
                        in1=dcol_sb[:, cc0:cc0 + ncc, :]
                            .to_broadcast([128, ncc, 256]),
                        op=mybir.AluOpType.is_equal)
                    for i, (cc, pp, kk, r) in enumerate(p.pieces[(j, b)]):
                        first, last = fl[r]
                        nc.tensor.matmul(
                            ps[r][:], lhsT=stag[pp:pp + kk, cc, :],
                            rhs=st[pp:pp + kk, cc, :],
                            start=(first == (b, i)), stop=(last == (b, i)),
                            tile_position=((96, 0) if pp == 96 else None))
                for r in range(2):
                    cg = 2 * j + r
                    first = fl[r][0]
                    cols = slice(256 * cg, 256 * (cg + 1))
                    t2 = evpool.tile([128, 256], f32, tag="ev2", name="ev2t")
                    if first is not None:
                        t1 = evpool.tile([128, 256], f32, tag="ev1",
                                         name="ev1t")
                        nc.vector.tensor_tensor(
                            out=t1[:], in0=ps[r][:], in1=hT[:, cols],
                            op=mybir.AluOpType.add)
                        nc.vector.tensor_tensor(
                            out=t2[:], in0=t1[:], in1=dinv[:, cols],
                            op=mybir.AluOpType.mult)
                    else:
                        nc.vector.tensor_tensor(
                            out=t2[:], in0=hT[:, cols], in1=dinv[:, cols],
                            op=mybir.AluOpType.mult)
                    nc.scalar.activation(
                        out=hT[:, cols], in_=t2[:],
                        func=mybir.ActivationFunctionType.Relu,
                        scale=AB_sb[:, 2 * l:2 * l + 1],
                        bias=AB_sb[:, 2 * l + 1:2 * l + 2])

            # layer 0 dense phase up front; later layers interleave with the
            # previous layer's aggregation (stripe k ready after cg 4k+3)
            for k in range(8):
                emit_z_stripe(0, k)
            for l in range(L):
                knext = 0            # next stripe of layer l+1 to emit
                for j in range(p.NPAIR):
                    emit_agg_pair(l, j)
                    if l == L - 1 and p.slot * (j + 1) <= P_own:
                        # pair j's 512 cols == graph j: pool it immediately
                        pcols = slice(p.slot * j, p.slot * (j + 1))
                        for q in range(2):
                            nc.vector.tensor_reduce(
                                out=gloc[:, gpc * q + j:gpc * q + j + 1],
                                in_=hT[:, pcols], axis=mybir.AxisListType.X,
                                op=pops[q])
                    while (l < L - 1 and knext < 8
                           and (2 * j + 2) * 256 >= (knext + 1) * STRIPE):
                        emit_z_stripe(l + 1, knext)
                        knext += 1

        # ---- pooling + MLP (pools released above) ----
        with contextlib.ExitStack() as ctx2:
            ppool = ctx2.enter_context(tc.tile_pool(name="poolp", bufs=1))
            mpool = ctx2.enter_context(
                tc.tile_pool(name="mlpp", bufs=2, space="PSUM"))
            nc.sync.dma_start(out=gpool_in[:], in_=gloc[:])
            nc.gpsimd.collective_compute(
                "AllGather", mybir.AluOpType.bypass,
                replica_groups=groups, ins=[gpool_in[:]],
                outs=[gpool_out[:]])
            gall = []
            for q in range(2):
                gt = ppool.tile([H, B], f32, name=f"gall{q}")
                nc.sync.dma_start(
                    out=gt[:].rearrange("f (c j) -> f c j", c=n_cores),
                    in_=gpool_out[:, gpc * q:gpc * (q + 1)].rearrange(
                        "(c f) j -> f c j", c=n_cores))
                gall.append(gt)
            gsum, gmax = gall
            pc_sb = ppool.tile([H, B], f32, name="pc_sb")
            ci_sb = ppool.tile([H, B], f32, name="ci_sb")
            f1w = ppool.tile([H, 3, H], f32, name="f1w")
            f1b = ppool.tile([H, 1], f32, name="f1b")
            f2w = ppool.tile([H, H // 2], f32, name="f2w")
            f2b = ppool.tile([H // 2, 1], f32, name="f2b")
            f3w = ppool.tile([H // 2, 1], f32, name="f3w")
            for name, t in [("padcorr", pc_sb), ("cntinv", ci_sb),
                            ("fc1w", f1w), ("fc1b", f1b), ("fc2w", f2w),
                            ("fc2b", f2b), ("fc3w", f3w)]:
                nc.sync.dma_start(t[:], P[name][:])
            nc.vector.tensor_tensor(out=gsum[:], in0=gsum[:], in1=pc_sb[:],
                                    op=mybir.AluOpType.subtract)
            gmean = ppool.tile([H, B], f32, name="gmean")
            nc.vector.tensor_tensor(out=gmean[:], in0=gsum[:], in1=ci_sb[:],
                                    op=mybir.AluOpType.mult)
            mp1 = mpool.tile([H, B], f32, space="PSUM", name="mp1")
            for i, g in enumerate([gmean, gmax, gsum]):
                nc.tensor.matmul(mp1[:], lhsT=f1w[:, i, :], rhs=g[:],
                                 start=(i == 0), stop=(i == 2))
            m1 = ppool.tile([H, B], f32, name="m1")
            nc.scalar.activation(out=m1[:], in_=mp1[:],
                                 func=mybir.ActivationFunctionType.Relu,
                                 bias=f1b[:, 0:1])
            mp2 = mpool.tile([H // 2, B], f32, space="PSUM", name="mp2")
            nc.tensor.matmul(mp2[:], lhsT=f2w[:], rhs=m1[:], start=True,
                             stop=True)
            m2 = ppool.tile([H // 2, B], f32, name="m2")
            nc.scalar.activation(out=m2[:], in_=mp2[:],
                                 func=mybir.ActivationFunctionType.Relu,
                                 bias=f2b[:, 0:1])
            mp3 = mpool.tile([1, B], f32, space="PSUM", name="mp3")
            nc.tensor.matmul(mp3[:], lhsT=f3w[:], rhs=m2[:], start=True,
                             stop=True)
            ob = ppool.tile([1, B], f32, name="ob")
            nc.vector.tensor_scalar_add(ob[:], mp3[:], float(fc3b_val))
            nc.sync.dma_start(out=out_t[:], in_=ob[:])

    nc.compile()
    return nc


# ============================== entry point ================================

def run(inputs, N, B, n_cores=8, slot=512, L=4, sim=False, linearize=False):
    """Full kernel: plan, build, execute, return [B,1] output."""
    p = build_plan(inputs["edge_index"], inputs["batch"], N, B,
                   n_cores=n_cores, slot=slot, L=L)
    in_maps, fc3b = prepare_inputs(
        p, inputs["x"], inputs["conv_ws"], inputs["conv_bs"],
        inputs["bn_gamma"], inputs["bn_beta"], inputs["bn_mean"],
        inputs["bn_var"], inputs["fc1_w"], inputs["fc1_b"], inputs["fc2_w"],
        inputs["fc2_b"], inputs["fc3_w"], inputs["fc3_b"])
    nc = build_nc(p, fc3b, debug=sim, linearize=linearize)
    if sim:
        from concourse.bass_interp import MultiCoreSim
        ms = MultiCoreSim(nc, num_cores=n_cores)
        for c in range(n_cores):
            for k, v in in_maps[c].items():
                ms.cores[c].tensor(k)[:] = v
        ms.simulate()
        out = np.asarray(ms.cores[0].tensor("out"))
    else:
        from concourse.bass_utils import run_bass_kernel_spmd
        res = run_bass_kernel_spmd(nc, in_maps, list(range(n_cores)))
        out = res.results[0]["out"]
    return out.reshape(B, 1).astype(F32)


# ============================== harness entry ==============================

_N, _B, _L = 100000, 256, 4


def kernel(**inputs):
    """Full-input entry point: shards across 8 NeuronCores internally."""
    inputs = {k: np.asarray(v) for k, v in inputs.items()}
    out = run(inputs, N=_N, B=_B, n_cores=8, slot=512, L=_L, sim=False)
    return out.astype(np.float32)
